# revision 8
# baseline (speedup 1.0000x reference)
"""Attention-pooling kernel for Trainium2 (8 NeuronCores, SPMD data-parallel).

Computes, for x: [B, S, H] and w: [H, 1]:
    scores[b, s] = sum_h tanh(x[b, s, h]) * w[h]
    attn = softmax(scores, axis=s)
    out[b, h]   = sum_s attn[b, s] * x[b, s, h]

Sharding: data-parallel over batch B across 8 cores (32 batches/core),
w replicated. No inter-core communication; host concatenates the shards.

Memory-regime roofline: each core reads 64 MiB of x once (~200 us at the
~336 GB/s-while-busy HBM share measured on this part), so the goal is a
per-batch pipeline period equal to the 2 MiB x-load (~6.2 us) with every
compute engine under that.

Per-core dataflow (per batch b), s-tile t in [0, 32), s = p*32 + t:
  DMA   : x[b] -> SBUF slots [1:33] of a 33-slot tile (16 KB contiguous
          per partition; float32r view of the same bytes). Slot 0 is
          never written — see the matmul trick below.
  ACT   : tanh(x) -> energy in FP16 (fp16 keeps the DVE 16-bit 2x rate
          of bf16 but with 8x the mantissa — scores |.|<40 fit easily)
  DVE   : energy *= w (fp16, in place, 2x_1p), then the h-reduction as
          an fp16 TT add-tree (128->64->32->16) + one fp32 tensor_reduce
          over the last 16. A monolithic tensor_reduce has no DVE perf
          mode and costs 2x more. All score compute stays on DVE:
          any concurrent GPSIMD op grabs the shared SBUF port pair and
          fully blocks DVE 16-bit TTs (measured: a 0.9 us mul stretched
          to 4.4 us ending exactly at GPSIMD-op end).
  ACT   : ebuf = exp(scores) (float32r), accum_out -> rowsum [128, 1]
  PE    : context via fp32r M=1 matmuls (fast path needs moving free
          >= 256), ALL accumulating into ONE psum tile ps[0, 0:256]:
          matmul for s-tile t uses lhsT=ebuf[:, t] and rhs = xb slots
          (t, t+1) (s-tiles (t-1, t) — slot 0 holds junk), so the
          useful product e_t*x_t always lands in ps[0, 128:256] and the
          garbage e_t*x_{t-1} in ps[0, 0:128]. No cross-bank add needed.
  PE    : total = rowsum.T @ ones    [1, 1]
  DVE   : recip = 1/total;  ACT: out_row = ps[0,128:256] * recip; DMA
          out on the scalar HWDGE ring.

Software pipelining (a naive emission runs at ~8.2 us/batch because
ACT's in-order stream serializes tanh(b) -> DVE chain(b) -> exp(b) ->
tanh(b+1)). Consumers are deferred so every engine only waits on work
from previous iterations:
  iter i emits: DMA(i); ACT exp(i-2); ACT tanh(i); DVE recip(i-3) +
  score chain(i); PE matmuls(i-2); ACT out_row(i-3) + out DMA(i-3).
Softmax normalization is factored out of the weighted sum (exp without
max-subtraction is safe: |scores| < ~40 here).
"""

import numpy as np

import concourse.bass as bass
import concourse.tile as tile
from concourse import bacc, mybir
from concourse.bass_utils import run_bass_kernel_spmd

B, S, H = 256, 4096, 128
N_CORES = 8
B_SHARD = B // N_CORES  # 32
P = 128                 # SBUF partitions; also H
S_TILES = S // P        # 32  (s = p * S_TILES + t)
XSLOTS = S_TILES + 1    # slot 0 = junk pad for the shifted-pair matmul

F32 = mybir.dt.float32
F32R = mybir.dt.float32r
F16 = mybir.dt.float16

_nc_cache = None


def _build() -> bass.Bass:
    nc = bacc.Bacc(None, target_bir_lowering=False, enable_partition_id=False)

    x_ext = nc.declare_dram_parameter(
        "encoder_outputs", [B_SHARD, S, H], F32, isOutput=False
    )
    w_ext = nc.declare_dram_parameter(
        "attention_weights", [H, 1], F32, isOutput=False
    )
    out_ext = nc.declare_dram_parameter("out", [B_SHARD, H], F32, isOutput=True)

    with tile.TileContext(nc) as tc:
        with (
            tc.tile_pool(name="singles", bufs=1) as singles,
            tc.tile_pool(name="xpool", bufs=9) as xpool,
            tc.tile_pool(name="evpool", bufs=3) as evpool,
            tc.tile_pool(name="small", bufs=8) as small,
            tc.tile_pool(name="psum_ctx", bufs=4, space="PSUM") as psum_ctx_pool,
            tc.tile_pool(name="psum_tot", bufs=2, space="PSUM") as psum_tot_pool,
        ):
            # w broadcast across partitions: w_bcast[p, h] = w[h].  On the
            # scalar HWDGE ring so the sync ring's x loads start at t=0.
            w_bcast = singles.tile([P, H], F32)
            w_flat = w_ext[:].rearrange("h one -> (one h)")
            w_row = bass.AP(
                tensor=w_flat.tensor,
                offset=w_flat.offset,
                ap=[[0, P], w_flat.ap[0]],
            )
            nc.scalar.dma_start(out=w_bcast, in_=w_row)

            ones_col = singles.tile([P, 1], F32)
            nc.vector.memset(ones_col, 1.0)

            # w replicated along the tile axis in fp16 (DVE is the only
            # reader). Log-doubling: 6 copies instead of 32 so the fill
            # phase isn't serialized behind ~8 us of setup casts.
            w_rep = singles.tile([P, S_TILES, H], F16)
            nc.vector.tensor_copy(w_rep[:, 0, :], w_bcast)
            n = 1
            while n < S_TILES:
                m = min(n, S_TILES - n)
                nc.vector.tensor_copy(
                    w_rep[:, n : n + m, :], w_rep[:, 0:m, :]
                )
                n += m

            # [b, p, t, h] view of DRAM; partition p reads 16 KB contiguous
            xv = x_ext[:].rearrange("b (p t) h -> b p t h", p=P)

            st = [dict() for _ in range(B_SHARD)]

            def stage0(b, d):  # load into slots [1:33]; slot 0 stays junk
                d["xb"] = xb = xpool.tile([P, XSLOTS, H], F32R, tag="xb", name="xb")
                nc.sync.dma_start(out=xb[:, 1:XSLOTS, :], in_=xv[b].bitcast(F32R))

            def stage_exp(b, d):  # exp(scores) -> ebuf, rowsum (ACT)
                d["ebuf"] = ebuf = small.tile([P, S_TILES], F32R, tag="ebuf", name="ebuf")
                d["rowsum"] = rowsum = small.tile([P, 1], F32, tag="rowsum", name="rowsum")
                nc.scalar.activation(
                    out=ebuf,
                    in_=d["scores"],
                    func=mybir.ActivationFunctionType.Exp,
                    accum_out=rowsum,
                )

            def stage1(b, d):  # tanh -> fp16 energy (two halves)
                xbf = d["xb"].bitcast(F32)
                half = S_TILES // 2
                d["ev"] = ev = evpool.tile([P, S_TILES, H], F16, tag="ev", name="ev")
                nc.scalar.activation(
                    out=ev[:, 0:half, :],
                    in_=xbf[:, 1 : 1 + half, :],
                    func=mybir.ActivationFunctionType.Tanh,
                )
                nc.scalar.activation(
                    out=ev[:, half:, :],
                    in_=xbf[:, 1 + half : XSLOTS, :],
                    func=mybir.ActivationFunctionType.Tanh,
                )

            def stage_chain(b, d):  # DVE: mul + fp16 tree + fp32 reduce
                # The mul is split at the tanh half boundary so DVE starts
                # on mul_a as soon as tanh_a lands instead of waiting for
                # the full tanh. This decoupling is load-bearing: a single
                # fused mul re-forms the ACT->DVE serial cycle and costs
                # ~38 us end-to-end (measured 246 us vs 207 us).
                d["scores"] = small.tile([P, S_TILES], F32, tag="scores", name="scores")
                ev = d["ev"]
                half = S_TILES // 2
                nc.vector.tensor_mul(
                    ev[:, 0:half, :], ev[:, 0:half, :], w_rep[:, 0:half, :]
                )
                nc.vector.tensor_mul(
                    ev[:, half:, :], ev[:, half:, :], w_rep[:, half:, :]
                )
                nc.vector.tensor_add(ev[:, :, 0:64], ev[:, :, 0:64], ev[:, :, 64:128])
                nc.vector.tensor_add(ev[:, :, 0:32], ev[:, :, 0:32], ev[:, :, 32:64])
                nc.vector.tensor_add(ev[:, :, 0:16], ev[:, :, 0:16], ev[:, :, 16:32])
                nc.vector.tensor_reduce(
                    out=d["scores"],
                    in_=ev[:, :, 0:16],
                    axis=mybir.AxisListType.X,
                    op=mybir.AluOpType.add,
                )

            def stage4(b, d):  # fp32r shifted-pair matmuls, one psum bank
                xb, ebuf = d["xb"], d["ebuf"]
                ps = psum_ctx_pool.tile([1, 2 * H], F32, tag="ps")
                for t in range(S_TILES):
                    # rhs slots (t, t+1) = s-tiles (t-1, t); useful half
                    # e_t * x_t lands in ps[0, 128:256]
                    nc.tensor.matmul(
                        ps,
                        ebuf[:, t : t + 1],
                        xb[:, t : t + 2, :],
                        start=(t == 0),
                        stop=(t == S_TILES - 1),
                    )
                tot_ps = psum_tot_pool.tile([1, 1], F32)
                nc.tensor.matmul(
                    tot_ps, d["rowsum"], ones_col, start=True, stop=True
                )
                d["ps"], d["tot_ps"] = ps, tot_ps

            def stage5_dve(b, d):  # DVE: reciprocal of the softmax total
                recip = small.tile([1, 1], F32, tag="recip")
                nc.vector.reciprocal(out=recip, in_=d["tot_ps"])
                d["recip"] = recip

            def stage5_act(b, d):  # ACT: normalize + store
                out_row = small.tile([1, H], F32, tag="out_row")
                nc.scalar.activation(
                    out=out_row,
                    in_=d["ps"][0:1, H : 2 * H],
                    func=mybir.ActivationFunctionType.Copy,
                    scale=d["recip"],
                )
                nc.scalar.dma_start(out=out_ext[b : b + 1, :], in_=out_row)

            def live(j):
                return 0 <= j < B_SHARD

            for i in range(B_SHARD + 3):
                if live(i):
                    stage0(i, st[i])
                # ACT stream: exp first (inputs one iter old -> no stall)
                if live(i - 2):
                    stage_exp(i - 2, st[i - 2])
                if live(i):
                    stage1(i, st[i])
                # DVE stream: epilogue recip first, then own score chain
                if live(i - 3):
                    stage5_dve(i - 3, st[i - 3])
                if live(i):
                    stage_chain(i, st[i])
                if live(i - 2):
                    stage4(i - 2, st[i - 2])
                if live(i - 3):
                    stage5_act(i - 3, st[i - 3])

    # Bacc pipeline: splits multi-sem waits (HW allows one per instr),
    # inserts GPSIMD library loads + ACT table loads, lowers extended ISA.
    nc.compile()
    return nc


def _get_nc() -> bass.Bass:
    global _nc_cache
    if _nc_cache is None:
        _nc_cache = _build()
    return _nc_cache


def run(encoder_outputs: np.ndarray, attention_weights: np.ndarray, **spmd_kwargs):
    """Run the SPMD kernel; returns (output [B, H], BassKernelResults)."""
    nc = _get_nc()
    x = np.ascontiguousarray(encoder_outputs, dtype=np.float32)
    w = np.ascontiguousarray(attention_weights, dtype=np.float32)
    assert x.shape == (B, S, H), x.shape
    assert w.shape == (H, 1), w.shape
    in_maps = [
        {
            "encoder_outputs": x[i * B_SHARD : (i + 1) * B_SHARD],
            "attention_weights": w,
        }
        for i in range(N_CORES)
    ]
    res = run_bass_kernel_spmd(nc, in_maps, core_ids=list(range(N_CORES)), **spmd_kwargs)
    out = np.concatenate(
        [res.results[i]["out"] for i in range(N_CORES)], axis=0
    ).astype(np.float32)
    return out, res


def kernel(encoder_outputs: np.ndarray, attention_weights: np.ndarray) -> np.ndarray:
    out, _ = run(encoder_outputs, attention_weights)
    return out
